# revision 49
# baseline (speedup 1.0000x reference)
"""Complex-valued multi-head attention on 8 Trainium2 NeuronCores.

Sharding: batch(2) x head-pairs(4) -> 8 cores; each core runs one batch
element and 2 heads end-to-end (QKV proj -> complex scores -> |s| softmax
-> AV -> partial W_O), host sums the W_O partials over the 4 cores of each
batch element (tensor-parallel reduce) and transposes to the output layout.

All matmuls fp16 (fp32 PSUM).  K projections (A=[kr;ki], C=[-ki;kr]) run
first over all column blocks so chunk-0 scores unlock ~30us in; q/V
projections are emitted later as PE filler.  V is projected directly in
transposed [k,dv] layout by swapping matmul operands (x block stationary),
removing the PE transposes and their evacuation copies.  Per score tile
[128k x 1024]: Pool squares s_re (and some s_im) straight out of PSUM into
f16 SBUF, DVE squares the rest and does the f16 adds plus a running f16
accumulation of the probabilities - the rowsum is one ones^T @ pacc matmul
per chunk instead of 32 ones-column matmuls.  Act runs only the batched
sqrt and exp passes (one 16k-column batch per chunk; chunk 0's sqrt in two
halves for an earlier start), so table sets swap just twice per chunk.
AV/normalize/W_O for chunk i are emitted during chunk i+1; W_O partials
are stored and DMA'd in fp16 and summed on the host in float64.
"""
import sys

sys.path.insert(0, "/opt/trn_rl_repo")

import numpy as np

B, NQ, NK, R = 2, 2048, 2048, 512
H, DK, DV = 8, 64, 64
NCORES = 8
NCC = 8          # n-chunks for projection streaming (2048/256)
NCW = 256        # projection n-chunk width
QC = 4           # q-chunks in attention (2048/512)
QCW = 512
KT = 16          # k-tiles (2048/128)

_CACHE = {}


def _build_nc():
    import concourse.bass as bass
    import concourse.tile as tile
    from concourse import bacc, mybir

    f32 = mybir.dt.float32
    f32r = mybir.dt.float32r
    f16 = mybir.dt.float16
    ALU = mybir.AluOpType
    AF = mybir.ActivationFunctionType

    nc = bacc.Bacc("TRN2", target_bir_lowering=False, debug=False,
                   num_devices=NCORES)

    kx_e = nc.dram_tensor("kx", [NCC, 8, 128, NCW], f16,
                          kind="ExternalInput")
    qx_e = nc.dram_tensor("qx", [NCC, 8, 128, NCW], f16,
                          kind="ExternalInput")
    vx_e = nc.dram_tensor("vx", [NCC, 8, 128, NCW], f16,
                          kind="ExternalInput")
    wpack_e = nc.dram_tensor("wpack", [128, 64 * 128], f16,
                             kind="ExternalInput")
    wopack_e = nc.dram_tensor("wopack", [128, 3 * 512], f16,
                              kind="ExternalInput")
    onesr_e = nc.dram_tensor("onesr", [1, 128], f32r, kind="ExternalInput")
    ore_e = nc.dram_tensor("out_re", [512, NQ], f16, kind="ExternalOutput")
    oim_e = nc.dram_tensor("out_im", [512, NQ], f16, kind="ExternalOutput")

    with tile.TileContext(nc) as tc:
      with nc.allow_low_precision(reason="fp16 softmax path"):
        with tc.tile_pool(name="pers", bufs=1) as pers, \
             tc.tile_pool(name="work", bufs=2) as work, \
             tc.tile_pool(name="psA", bufs=1, space="PSUM") as psA:

            # ---- constants (A/C weight slice first so PE starts early) ----
            wp = pers.tile([128, 64 * 128], f16, tag="wp")
            AC_LO, AC_HI = 4 * 4 * 128, 12 * 4 * 128
            nc.sync.dma_start(wp[:, AC_LO:AC_HI], wpack_e[:, AC_LO:AC_HI])
            ones_row = pers.tile([1, 128], f32r, tag="ones_row")
            nc.sync.dma_start(ones_row[:], onesr_e[:])
            ones16 = pers.tile([128, 1], f16, tag="ones16")
            nc.vector.memset(ones16[:], 1.0)
            eb_exp = pers.tile([128, 1], f32, tag="eb_exp")
            nc.vector.memset(eb_exp[:], -1.5)          # exp(mag - 1.5)

            qxts = {}
            vxts = {}

            def dma_q(j):
                qxt = work.tile([128, 8 * NCW], f16, tag="qxt", bufs=1,
                                name=f"qxt_{j}")
                nc.sync.dma_start(
                    qxt[:].rearrange("p (b f) -> p b f", f=NCW),
                    qx_e[j].rearrange("b p f -> p b f"))
                qxts[j] = qxt

            def dma_v(j):
                vxt = work.tile([128, 8 * NCW], f16, tag="vxt", bufs=2,
                                name=f"vxt_{j}")
                nc.sync.dma_start(
                    vxt[:].rearrange("p (b f) -> p b f", f=NCW),
                    vx_e[j].rearrange("b p f -> p b f"))
                vxts[j] = vxt

            # k-block input DMAs up front
            kxts = []
            for j in range(NCC):
                if j == 1:
                    nc.sync.dma_start(wp[:, 0:AC_LO], wpack_e[:, 0:AC_LO])
                kxt = work.tile([128, 8 * NCW], f16, tag="kxt", bufs=2,
                                name=f"kxt_{j}")
                nc.sync.dma_start(
                    kxt[:].rearrange("p (b f) -> p b f", f=NCW),
                    kx_e[j].rearrange("b p f -> p b f"))
                kxts.append(kxt)
                if j == 2:
                    # v weights ride between the k blocks
                    nc.sync.dma_start(wp[:, AC_HI:], wpack_e[:, AC_HI:])
                if j >= 2:
                    dma_v(j - 2)
            dma_v(6)
            dma_v(7)
            wop = pers.tile([128, 3 * 512], f16, tag="wop")
            nc.sync.dma_start(wop[:], wopack_e[:])

            def wblk(w, rc):
                return wp[:, (w * 4 + rc) * 128:(w * 4 + rc + 1) * 128]

            # ---- projection destinations (fp16) ----
            q_sb = [pers.tile([128, NQ], f16, tag=f"q_sb{h}",
                              name=f"q_sb{h}") for h in (0, 1)]
            A_sb = [pers.tile([128, NK], f16, tag=f"A_sb{h}",
                              name=f"A_sb{h}") for h in (0, 1)]
            C_sb = [pers.tile([128, NK], f16, tag=f"C_sb{h}",
                              name=f"C_sb{h}") for h in (0, 1)]
            v16_h = [pers.tile([128, NK], f16, tag=f"v16_h{h}",
                               name=f"v16_h{h}") for h in (0, 1)]

            oT_re = pers.tile([128, NQ], f16, tag="oT_re")
            oT_im = pers.tile([128, NQ], f16, tag="oT_im")

            # round-robin engines for projection PSUM evacuation.
            # Act participates only while otherwise idle (the AC prologue);
            # later copies would jam the Act queue ahead of sqrt/exp.
            cp_engines = [[nc.scalar, nc.vector],
                          [nc.vector]]
            cp_state = [0, 0]

            def cp_copy(dst, src, phase=1):
                engs = cp_engines[phase]
                eng = engs[cp_state[phase] % len(engs)]
                cp_state[phase] += 1
                if eng is nc.scalar:
                    eng.copy(dst, src)
                else:
                    eng.tensor_copy(dst, src)

            def emit_proj_AC(j):
                kxt = kxts[j]
                cs = slice(j * NCW, (j + 1) * NCW)
                # dests A0,A1,C0,C1 <- weight blocks (4,5),(6,7),(8,9),(10,11)
                for di, dest in enumerate([A_sb[0], A_sb[1],
                                           C_sb[0], C_sb[1]]):
                    pj = psA.tile([128, NCW], f32, tag="s_re", bufs=2,
                                  name=f"pjk_{j}_{di}")
                    w0 = 4 + 2 * di
                    for rc in range(4):
                        nc.tensor.matmul(
                            pj[:], wblk(w0, rc),
                            kxt[:, rc * NCW:(rc + 1) * NCW],
                            start=(rc == 0), stop=False)
                    for rc in range(4):
                        nc.tensor.matmul(
                            pj[:], wblk(w0 + 1, rc),
                            kxt[:, (4 + rc) * NCW:(5 + rc) * NCW],
                            start=False, stop=(rc == 3))
                    cp_copy(dest[:, cs], pj[:], phase=0)

            def emit_proj_q(j):
                qxt = qxts.pop(j)
                cs = slice(j * NCW, (j + 1) * NCW)
                for hh, dest in enumerate(q_sb):
                    pj = psA.tile([128, NCW], f32, tag="s_re", bufs=2,
                                  name=f"pjq_{j}_{hh}")
                    for rc in range(4):
                        nc.tensor.matmul(
                            pj[:], wblk(2 * hh, rc),
                            qxt[:, rc * NCW:(rc + 1) * NCW],
                            start=(rc == 0), stop=False)
                    for rc in range(4):
                        nc.tensor.matmul(
                            pj[:], wblk(2 * hh + 1, rc),
                            qxt[:, (4 + rc) * NCW:(5 + rc) * NCW],
                            start=False, stop=(rc == 3))
                    cp_copy(dest[:, cs], pj[:])

            def emit_proj_v(j):
                vxt = vxts.pop(j)
                for h in (0, 1):
                    for nb in (0, 1):
                        vj = psA.tile([128, 128], f32, tag="oAV",
                                      bufs=2, name=f"vj_{j}_{h}_{nb}")
                        for rc in range(4):
                            c0 = rc * NCW + nb * 128
                            nc.tensor.matmul(vj[:], vxt[:, c0:c0 + 128],
                                             wblk(12 + 2 * h, rc),
                                             start=(rc == 0), stop=False)
                        for rc in range(4):
                            c0 = (4 + rc) * NCW + nb * 128
                            nc.tensor.matmul(vj[:], vxt[:, c0:c0 + 128],
                                             wblk(13 + 2 * h, rc),
                                             start=False, stop=(rc == 3))
                        kb = (2 * j + nb) * 128
                        cp_copy(v16_h[h][:, kb:kb + 128], vj[:])

            # ---- attention pieces ----
            def emit_scores_kt(qc, kt, bt, blk0=0):
                # GPSIMD cannot read PSUM: squares run on Act/DVE, the
                # f16 adds and pacc run on Pool (SBUF only).
                qs = slice(qc * QCW, (qc + 1) * QCW)
                ks = slice(kt * 128, (kt + 1) * 128)
                kb = kt - blk0
                s_re = psA.tile([128, 1024], f32, tag="s_re", bufs=2,
                                name=f"s_re_{qc}_{kt}")
                s_im = psA.tile([128, 1024], f32, tag="s_im", bufs=1,
                                name=f"s_im_{qc}_{kt}")
                for h in (0, 1):
                    col = slice(h * 512, h * 512 + 512)
                    nc.tensor.matmul(s_re[:, col], A_sb[h][:, ks],
                                     q_sb[h][:, qs], start=True, stop=True)
                    nc.tensor.matmul(s_im[:, col], C_sb[h][:, ks],
                                     q_sb[h][:, qs], start=True, stop=True)
                sqre = work.tile([128, 1024], f16, tag="sqre", bufs=2,
                                 name=f"sqre_{qc}_{kt}")
                if qc == 0:
                    # Act squares hide inside the projection prologue
                    nc.scalar.square(sqre[:], s_re[:])
                else:
                    # hw: only one PSUM input per op, so copy then square
                    t16 = work.tile([128, 1024], f16, tag="t16", bufs=1,
                                    name=f"t16_{qc}_{kt}")
                    nc.vector.tensor_copy(t16[:], s_re[:])
                    nc.gpsimd.tensor_tensor(sqre[:], t16[:], t16[:],
                                            ALU.mult)
                sqim = work.tile([128, 1024], f16, tag="sqim", bufs=2,
                                 name=f"sqim_{qc}_{kt}")
                if qc == 0:
                    nc.scalar.square(sqim[:], s_im[:])
                else:
                    ti16 = work.tile([128, 1024], f16, tag="ti16", bufs=1,
                                     name=f"ti16_{qc}_{kt}")
                    nc.vector.tensor_copy(ti16[:], s_im[:])
                    nc.vector.tensor_mul(sqim[:], ti16[:], ti16[:])
                nc.gpsimd.tensor_tensor(bt[:, kb * 1024:(kb + 1) * 1024],
                                        sqre[:], sqim[:], ALU.add)

            def new_batch_tile(qc):
                return work.tile([128, 16 * 1024], f16, tag="batch", bufs=2,
                                 name=f"bt_{qc}")

            def new_half_tile(qc, half):
                return work.tile([128, 8 * 1024], f16, tag="batch3", bufs=2,
                                 name=f"bt_{qc}_{half}")

            def emit_sqrt(bt):
                nc.scalar.activation(bt[:], bt[:], AF.Sqrt,
                                     scale=1.0 / 64.0)

            def emit_av(bt, o_ps, qc, kts=range(KT), blk0=0):
                for kt in kts:
                    k8 = kt - blk0
                    for h in (0, 1):
                        col = slice(k8 * 1024 + h * 512,
                                    k8 * 1024 + h * 512 + 512)
                        vblk = v16_h[h][:, kt * 128:(kt + 1) * 128]
                        nc.tensor.matmul(o_ps[h][:, :], vblk, bt[:, col],
                                         start=(kt == 0),
                                         stop=(kt == KT - 1))

            def start_av(qc):
                return [psA.tile([128, QCW], f32, tag="oAV", bufs=2,
                                 name=f"o{h}_{qc}") for h in (0, 1)]

            def emit_norm(o_ps, paccs, qc):
                qs = slice(qc * QCW, (qc + 1) * QCW)
                rss = []
                for h in (0, 1):
                    rs = psA.tile([1, 512], f32, tag="s_im", bufs=1,
                                  name=f"rs_{qc}_{h}")
                    for pi, pc in enumerate(paccs):
                        nc.tensor.matmul(rs[:], ones16[:],
                                         pc[:, h * 512:h * 512 + 512],
                                         start=(pi == 0),
                                         stop=(pi == len(paccs) - 1))
                    rss.append(rs)
                bc_sb = work.tile([128, 1024], f32r, tag="bc_sb", bufs=1,
                                  name=f"bc_sb_{qc}")
                for h in (0, 1):
                    recip = work.tile([1, QCW], f32r, tag=f"recip{h}",
                                      bufs=1, name=f"recip{h}_{qc}")
                    nc.vector.reciprocal(recip[:], rss[h][:])
                    bc = psA.tile([128, 512], f32, tag="s_im", bufs=1,
                                  name=f"bc_{qc}_{h}")
                    nc.tensor.matmul(bc[:], ones_row[:], recip[:],
                                     start=True, stop=True)
                    nc.vector.tensor_copy(
                        bc_sb[:, h * 512:h * 512 + 512], bc[:])
                for h in (0, 1):
                    o_sb = work.tile([128, QCW], f16, tag="o_sb", bufs=1,
                                     name=f"o_sb_{qc}_{h}")
                    nc.scalar.copy(o_sb[:], o_ps[h][:])
                    for ri, dest in ((0, oT_re), (1, oT_im)):
                        rows = slice(64 * ri, 64 * ri + 64)
                        nc.vector.scalar_tensor_tensor(
                            dest[64 * h:64 * h + 64, qs],
                            o_sb[rows, :], 1.0,
                            bc_sb[rows, h * 512:h * 512 + 512],
                            ALU.mult, ALU.mult)

            def emit_wo(qc, rc_lo=0, rc_hi=4):
                qs = slice(qc * QCW, (qc + 1) * QCW)
                for Rc in range(rc_lo, rc_hi):
                    wo_re = psA.tile([128, QCW], f32, tag="oAV", bufs=2,
                                     name=f"wore_{Rc}_{qc}")
                    wo_im = psA.tile([128, QCW], f32, tag="oAV", bufs=2,
                                     name=f"woim_{Rc}_{qc}")

                    def wob(w):
                        return wop[:, w * 512 + Rc * 128:
                                   w * 512 + Rc * 128 + 128]

                    nc.tensor.matmul(wo_re[:], wob(0), oT_re[:, qs],
                                     start=True, stop=False)
                    nc.tensor.matmul(wo_re[:], wob(2), oT_im[:, qs],
                                     start=False, stop=True)
                    nc.tensor.matmul(wo_im[:], wob(1), oT_re[:, qs],
                                     start=True, stop=False)
                    nc.tensor.matmul(wo_im[:], wob(0), oT_im[:, qs],
                                     start=False, stop=True)
                    st_re = work.tile([128, QCW], f16, tag="st_re", bufs=1,
                                      name=f"st_re_{Rc}_{qc}")
                    nc.scalar.copy(st_re[:], wo_re[:])
                    nc.sync.dma_start(
                        ore_e[Rc * 128:(Rc + 1) * 128, qs], st_re[:])
                    st_im = work.tile([128, QCW], f16, tag="st_im", bufs=1,
                                      name=f"st_im_{Rc}_{qc}")
                    nc.scalar.copy(st_im[:], wo_im[:])
                    nc.sync.dma_start(
                        oim_e[Rc * 128:(Rc + 1) * 128, qs], st_im[:])

            def emit_exp(bt):
                nc.scalar.activation(bt[:], bt[:], AF.Exp, bias=eb_exp[:])

            def emit_pacc_half(qc, bt, a, blk0, eng=None, nblk=8):
                eng = eng or nc.vector
                pacc = work.tile([128, 1024], f16, tag=f"pacc{a}",
                                 bufs=1, name=f"pacc{a}_{qc}")
                eng.tensor_tensor(
                    pacc[:], bt[:, blk0 * 1024:(blk0 + 1) * 1024],
                    bt[:, (blk0 + 1) * 1024:(blk0 + 2) * 1024], ALU.add)
                for k8 in range(blk0 + 2, blk0 + nblk):
                    eng.tensor_tensor(
                        pacc[:], pacc[:],
                        bt[:, k8 * 1024:(k8 + 1) * 1024], ALU.add)
                return pacc

            def emit_pacc(qc, bt):
                # two independent accumulators so the rowsum matmul can
                # start after the first half
                return [emit_pacc_half(qc, bt, 0, 0, eng=nc.gpsimd),
                        emit_pacc_half(qc, bt, 1, 8, eng=nc.gpsimd)]

            # ---- emission script ----
            # prologue: A/C projections interleaved with chunk-0 scores and
            # the v projections, so every engine spins up early and the
            # steady-state iterations stay slim.
            dma_q(0)
            dma_q(1)
            bt0 = new_batch_tile(0)
            emit_proj_AC(0)
            emit_proj_AC(1)
            emit_proj_q(0)
            emit_proj_q(1)
            for kt in range(4):
                emit_scores_kt(0, kt, bt0)
            for j in range(2, NCC):
                emit_proj_AC(j)
                emit_scores_kt(0, 2 * j, bt0)
                emit_scores_kt(0, 2 * j + 1, bt0)
                if j >= 2:
                    emit_proj_v(j - 2)
                if j == 2:
                    dma_q(2)
                    dma_q(3)
            emit_sqrt(bt0)
            emit_exp(bt0)
            emit_proj_q(2)
            emit_proj_q(3)
            dma_q(4)
            dma_q(5)

            # steady pipeline: iter qc emits [WO(qc-2) fillers]
            # scores/sqrt/exp(qc), q-proj filler, then AV/pacc/norm(qc-1)
            # at the end (exp(qc-1) is finished well before).
            av_pend = (bt0, 0)     # exp'd, needs AV+pacc+norm
            wo_pend = None         # normalized, needs W_O
            for qc in range(1, QC):
                last = qc == QC - 1
                if last:
                    bth = [new_half_tile(qc, 0), new_half_tile(qc, 1)]
                else:
                    bt = new_batch_tile(qc)

                fillers = []
                if wo_pend is not None:
                    for rc in range(4):
                        fillers.append(
                            lambda rc=rc: emit_wo(wo_pend, rc, rc + 1))
                fi = 0
                for kt in range(KT):
                    emit_scores_kt(qc, kt,
                                   bth[kt // 8] if last else bt,
                                   (kt // 8) * 8 if last else 0)
                    if last and kt == 7:
                        emit_sqrt(bth[0])
                    if kt % 2 == 1 and fi < len(fillers):
                        fillers[fi]()
                        fi += 1
                while fi < len(fillers):
                    fillers[fi]()
                    fi += 1
                if last:
                    emit_sqrt(bth[1])
                else:
                    emit_sqrt(bt)
                    emit_exp(bt)
                if qc == 1:
                    emit_proj_v(6)
                    emit_proj_v(7)
                    emit_proj_q(4)
                    emit_proj_q(5)
                    dma_q(6)
                    dma_q(7)
                elif qc == 2:
                    emit_proj_q(6)
                    emit_proj_q(7)
                pbt, pqc2 = av_pend
                o_ps_av = start_av(pqc2)
                emit_av(pbt, o_ps_av, pqc2)
                paccs_new = emit_pacc(pqc2, pbt)
                emit_norm(o_ps_av, paccs_new, pqc2)
                wo_pend = pqc2
                if not last:
                    av_pend = (bt, qc)

            # drain chunk 3: exp/AV/pacc per half, then norm/WO
            emit_wo(wo_pend, 0, 4)
            o_ps3 = start_av(3)
            emit_exp(bth[0])
            emit_av(bth[0], o_ps3, 3, range(0, 8), 0)
            pacc3a = emit_pacc_half(3, bth[0], 0, 0)
            emit_exp(bth[1])
            emit_av(bth[1], o_ps3, 3, range(8, 16), 8)
            pacc3b = emit_pacc_half(3, bth[1], 1, 0, eng=nc.gpsimd, nblk=4)
            pacc3c = emit_pacc_half(3, bth[1], 2, 4, eng=nc.vector, nblk=4)
            emit_norm(o_ps3, [pacc3a, pacc3b, pacc3c], 3)
            emit_wo(3, 0, 4)

    nc.finalize()
    return nc



def _get_nc():
    if "nc" not in _CACHE:
        _CACHE["nc"] = _build_nc()
    return _CACHE["nc"]


def _core_inputs(c, inputs):
    b = c // 4
    h0 = 2 * (c % 4)
    hs = slice(h0 * 64, h0 * 64 + 128)

    def xpack_of(names):
        out = np.empty((NCC, 4 * len(names), 128, NCW), np.float16)
        for t, name in enumerate(names):
            xT = np.ascontiguousarray(inputs[name][b].T)      # (512, 2048)
            out[:, t * 4:(t + 1) * 4] = (
                xT.reshape(4, 128, NCC, NCW).transpose(2, 0, 1, 3))
        return out

    kx = xpack_of(("K_real", "K_imag"))
    qx = xpack_of(("Q_real", "Q_imag"))
    vx = xpack_of(("V_real", "V_imag"))

    wlist = []
    for kind in ("q", "A", "C"):
        base_r = inputs[{"q": "wq_r", "A": "wk_r", "C": "wk_r"}[kind]]
        base_i = inputs[{"q": "wq_i", "A": "wk_i", "C": "wk_i"}[kind]]
        for hh in (0, 1):
            rows = slice((h0 + hh) * 64, (h0 + hh) * 64 + 64)
            wr, wi_ = base_r[rows], base_i[rows]
            if kind == "C":
                w1 = np.vstack([-wi_, wr])       # x_re weights
                w2 = np.vstack([-wr, -wi_])      # x_im weights
            else:
                w1 = np.vstack([wr, wi_])
                w2 = np.vstack([-wi_, wr])
            wlist += [w1, w2]
    # v (transposed layout): moving weights [R, 128] = [wv_r.T | wv_i.T]
    # for the x_re passes and [-wv_i.T | wv_r.T] for the x_im passes.
    for hh in (0, 1):
        rows = slice((h0 + hh) * 64, (h0 + hh) * 64 + 64)
        wvr = inputs["wv_r"][rows]               # (64, 512)
        wvi = inputs["wv_i"][rows]
        W1 = np.concatenate([wvr.T, wvi.T], axis=1)      # (512, 128)
        W2 = np.concatenate([-wvi.T, wvr.T], axis=1)
        wlist += [W1.T, W2.T]                    # stored as (out, in)

    arr = np.empty((64, 128, 128), np.float16)
    for wi, mat in enumerate(wlist):
        arr[wi * 4:(wi + 1) * 4] = np.ascontiguousarray(mat.T).reshape(
            4, 128, 128)
    wpack = np.ascontiguousarray(arr.transpose(1, 0, 2)).reshape(
        128, 64 * 128)

    wo_r_T = np.ascontiguousarray(inputs["wo_r"][:, hs].T)    # (128, 512)
    wo_i_T = np.ascontiguousarray(inputs["wo_i"][:, hs].T)
    wopack = np.concatenate([wo_r_T, wo_i_T, -wo_i_T], axis=1)
    wopack = np.ascontiguousarray(wopack).astype(np.float16)

    return {
        "kx": kx,
        "qx": qx,
        "vx": vx,
        "wpack": wpack,
        "wopack": wopack,
        "onesr": np.ones((1, 128), np.float32),
    }


def kernel(**inputs):
    from concourse.bass_utils import run_bass_kernel_spmd

    nc = _get_nc()
    in_maps = [_core_inputs(c, inputs) for c in range(NCORES)]
    res = run_bass_kernel_spmd(nc, in_maps, list(range(NCORES)))
    out = np.empty((B, NQ, R, 2), np.float32)
    for b in range(B):
        re = np.zeros((512, NQ), np.float64)
        im = np.zeros((512, NQ), np.float64)
        for c in range(b * 4, b * 4 + 4):
            re += res.results[c]["out_re"].astype(np.float64)
            im += res.results[c]["out_im"].astype(np.float64)
        out[b, :, :, 0] = re.T
        out[b, :, :, 1] = im.T
    return out


# revision 50
# speedup vs baseline: 1.0310x; 1.0310x over previous
"""Complex-valued multi-head attention on 8 Trainium2 NeuronCores.

Sharding: batch(2) x head-pairs(4) -> 8 cores; each core runs one batch
element and 2 heads end-to-end (QKV proj -> complex scores -> |s| softmax
-> AV -> partial W_O), host sums the W_O partials over the 4 cores of each
batch element (tensor-parallel reduce) and transposes to the output layout.

All matmuls fp16 (fp32 PSUM).  K projections (A=[kr;ki], C=[-ki;kr]) run
first over all column blocks so chunk-0 scores unlock ~30us in; q/V
projections are emitted later as PE filler.  V is projected directly in
transposed [k,dv] layout by swapping matmul operands (x block stationary),
removing the PE transposes and their evacuation copies.  Per score tile
[128k x 1024]: Pool squares s_re (and some s_im) straight out of PSUM into
f16 SBUF, DVE squares the rest and does the f16 adds plus a running f16
accumulation of the probabilities - the rowsum is one ones^T @ pacc matmul
per chunk instead of 32 ones-column matmuls.  Act runs only the batched
sqrt and exp passes (one 16k-column batch per chunk; chunk 0's sqrt in two
halves for an earlier start), so table sets swap just twice per chunk.
AV/normalize/W_O for chunk i are emitted during chunk i+1; W_O partials
are stored and DMA'd in fp16 and summed on the host in float64.
"""
import sys

sys.path.insert(0, "/opt/trn_rl_repo")

import numpy as np

B, NQ, NK, R = 2, 2048, 2048, 512
H, DK, DV = 8, 64, 64
NCORES = 8
NCC = 8          # n-chunks for projection streaming (2048/256)
NCW = 256        # projection n-chunk width
QC = 4           # q-chunks in attention (2048/512)
QCW = 512
KT = 16          # k-tiles (2048/128)

_CACHE = {}


def _build_nc():
    import concourse.bass as bass
    import concourse.tile as tile
    from concourse import bacc, mybir

    f32 = mybir.dt.float32
    f32r = mybir.dt.float32r
    f16 = mybir.dt.float16
    ALU = mybir.AluOpType
    AF = mybir.ActivationFunctionType

    nc = bacc.Bacc("TRN2", target_bir_lowering=False, debug=False,
                   num_devices=NCORES)

    kx_e = nc.dram_tensor("kx", [NCC, 8, 128, NCW], f16,
                          kind="ExternalInput")
    qx_e = nc.dram_tensor("qx", [NCC, 8, 128, NCW], f16,
                          kind="ExternalInput")
    vx_e = nc.dram_tensor("vx", [NCC, 8, 128, NCW], f16,
                          kind="ExternalInput")
    wpack_e = nc.dram_tensor("wpack", [128, 64 * 128], f16,
                             kind="ExternalInput")
    wopack_e = nc.dram_tensor("wopack", [128, 3 * 512], f16,
                              kind="ExternalInput")
    onesr_e = nc.dram_tensor("onesr", [1, 128], f32r, kind="ExternalInput")
    ore_e = nc.dram_tensor("out_re", [512, NQ], f16, kind="ExternalOutput")
    oim_e = nc.dram_tensor("out_im", [512, NQ], f16, kind="ExternalOutput")

    with tile.TileContext(nc) as tc:
      with nc.allow_low_precision(reason="fp16 softmax path"):
        with tc.tile_pool(name="pers", bufs=1) as pers, \
             tc.tile_pool(name="work", bufs=2) as work, \
             tc.tile_pool(name="psA", bufs=1, space="PSUM") as psA:

            # ---- constants (A/C weight slice first so PE starts early) ----
            wp = pers.tile([128, 64 * 128], f16, tag="wp")
            AC_LO, AC_HI = 4 * 4 * 128, 12 * 4 * 128
            nc.sync.dma_start(wp[:, AC_LO:AC_HI], wpack_e[:, AC_LO:AC_HI])
            ones_row = pers.tile([1, 128], f32r, tag="ones_row")
            nc.sync.dma_start(ones_row[:], onesr_e[:])
            ones16 = pers.tile([128, 1], f16, tag="ones16")
            nc.vector.memset(ones16[:], 1.0)
            eb_exp = pers.tile([128, 1], f32, tag="eb_exp")
            nc.vector.memset(eb_exp[:], -1.5)          # exp(mag - 1.5)

            qxts = {}
            vxts = {}

            def dma_q(j):
                qxt = work.tile([128, 8 * NCW], f16, tag="qxt", bufs=1,
                                name=f"qxt_{j}")
                nc.sync.dma_start(
                    qxt[:].rearrange("p (b f) -> p b f", f=NCW),
                    qx_e[j].rearrange("b p f -> p b f"))
                qxts[j] = qxt

            def dma_v(j):
                vxt = work.tile([128, 8 * NCW], f16, tag="vxt", bufs=2,
                                name=f"vxt_{j}")
                nc.sync.dma_start(
                    vxt[:].rearrange("p (b f) -> p b f", f=NCW),
                    vx_e[j].rearrange("b p f -> p b f"))
                vxts[j] = vxt

            # k-block input DMAs up front
            kxts = []
            for j in range(NCC):
                if j == 1:
                    nc.sync.dma_start(wp[:, 0:AC_LO], wpack_e[:, 0:AC_LO])
                kxt = work.tile([128, 8 * NCW], f16, tag="kxt", bufs=2,
                                name=f"kxt_{j}")
                nc.sync.dma_start(
                    kxt[:].rearrange("p (b f) -> p b f", f=NCW),
                    kx_e[j].rearrange("b p f -> p b f"))
                kxts.append(kxt)
                if j == 2:
                    # v weights ride between the k blocks
                    nc.sync.dma_start(wp[:, AC_HI:], wpack_e[:, AC_HI:])
                if j >= 2:
                    dma_v(j - 2)
            dma_v(6)
            dma_v(7)
            wop = pers.tile([128, 3 * 512], f16, tag="wop")
            nc.sync.dma_start(wop[:], wopack_e[:])

            def wblk(w, rc):
                return wp[:, (w * 4 + rc) * 128:(w * 4 + rc + 1) * 128]

            # ---- projection destinations (fp16) ----
            q_sb = [pers.tile([128, NQ], f16, tag=f"q_sb{h}",
                              name=f"q_sb{h}") for h in (0, 1)]
            A_sb = [pers.tile([128, NK], f16, tag=f"A_sb{h}",
                              name=f"A_sb{h}") for h in (0, 1)]
            C_sb = [pers.tile([128, NK], f16, tag=f"C_sb{h}",
                              name=f"C_sb{h}") for h in (0, 1)]
            v16_h = [pers.tile([128, NK], f16, tag=f"v16_h{h}",
                               name=f"v16_h{h}") for h in (0, 1)]

            oT_re = pers.tile([128, NQ], f16, tag="oT_re")
            oT_im = pers.tile([128, NQ], f16, tag="oT_im")

            # round-robin engines for projection PSUM evacuation.
            # Act participates only while otherwise idle (the AC prologue);
            # later copies would jam the Act queue ahead of sqrt/exp.
            cp_engines = [[nc.scalar, nc.vector],
                          [nc.vector]]
            cp_state = [0, 0]

            def cp_copy(dst, src, phase=1):
                engs = cp_engines[phase]
                eng = engs[cp_state[phase] % len(engs)]
                cp_state[phase] += 1
                if eng is nc.scalar:
                    eng.copy(dst, src)
                else:
                    eng.tensor_copy(dst, src)

            def emit_proj_AC(j):
                kxt = kxts[j]
                cs = slice(j * NCW, (j + 1) * NCW)
                # dests A0,A1,C0,C1 <- weight blocks (4,5),(6,7),(8,9),(10,11)
                for di, dest in enumerate([A_sb[0], A_sb[1],
                                           C_sb[0], C_sb[1]]):
                    pj = psA.tile([128, NCW], f32, tag="s_re", bufs=2,
                                  name=f"pjk_{j}_{di}")
                    w0 = 4 + 2 * di
                    for rc in range(4):
                        nc.tensor.matmul(
                            pj[:], wblk(w0, rc),
                            kxt[:, rc * NCW:(rc + 1) * NCW],
                            start=(rc == 0), stop=False)
                    for rc in range(4):
                        nc.tensor.matmul(
                            pj[:], wblk(w0 + 1, rc),
                            kxt[:, (4 + rc) * NCW:(5 + rc) * NCW],
                            start=False, stop=(rc == 3))
                    cp_copy(dest[:, cs], pj[:], phase=0)

            def emit_proj_q(j):
                qxt = qxts.pop(j)
                cs = slice(j * NCW, (j + 1) * NCW)
                for hh, dest in enumerate(q_sb):
                    pj = psA.tile([128, NCW], f32, tag="s_re", bufs=2,
                                  name=f"pjq_{j}_{hh}")
                    for rc in range(4):
                        nc.tensor.matmul(
                            pj[:], wblk(2 * hh, rc),
                            qxt[:, rc * NCW:(rc + 1) * NCW],
                            start=(rc == 0), stop=False)
                    for rc in range(4):
                        nc.tensor.matmul(
                            pj[:], wblk(2 * hh + 1, rc),
                            qxt[:, (4 + rc) * NCW:(5 + rc) * NCW],
                            start=False, stop=(rc == 3))
                    cp_copy(dest[:, cs], pj[:])

            def emit_proj_v(j):
                vxt = vxts.pop(j)
                for h in (0, 1):
                    for nb in (0, 1):
                        vj = psA.tile([128, 128], f32, tag="oAV",
                                      bufs=2, name=f"vj_{j}_{h}_{nb}")
                        for rc in range(4):
                            c0 = rc * NCW + nb * 128
                            nc.tensor.matmul(vj[:], vxt[:, c0:c0 + 128],
                                             wblk(12 + 2 * h, rc),
                                             start=(rc == 0), stop=False)
                        for rc in range(4):
                            c0 = (4 + rc) * NCW + nb * 128
                            nc.tensor.matmul(vj[:], vxt[:, c0:c0 + 128],
                                             wblk(13 + 2 * h, rc),
                                             start=False, stop=(rc == 3))
                        kb = (2 * j + nb) * 128
                        cp_copy(v16_h[h][:, kb:kb + 128], vj[:])

            # ---- attention pieces ----
            def emit_scores_kt(qc, kt, bt, blk0=0):
                # GPSIMD cannot read PSUM: squares run on Act/DVE, the
                # f16 adds and pacc run on Pool (SBUF only).
                qs = slice(qc * QCW, (qc + 1) * QCW)
                ks = slice(kt * 128, (kt + 1) * 128)
                kb = kt - blk0
                s_re = psA.tile([128, 1024], f32, tag="s_re", bufs=2,
                                name=f"s_re_{qc}_{kt}")
                s_im = psA.tile([128, 1024], f32, tag="s_im", bufs=1,
                                name=f"s_im_{qc}_{kt}")
                for h in (0, 1):
                    col = slice(h * 512, h * 512 + 512)
                    nc.tensor.matmul(s_re[:, col], A_sb[h][:, ks],
                                     q_sb[h][:, qs], start=True, stop=True)
                    nc.tensor.matmul(s_im[:, col], C_sb[h][:, ks],
                                     q_sb[h][:, qs], start=True, stop=True)
                sqre = work.tile([128, 1024], f16, tag="sqre", bufs=2,
                                 name=f"sqre_{qc}_{kt}")
                if qc == 0:
                    # Act squares hide inside the projection prologue
                    nc.scalar.square(sqre[:], s_re[:])
                else:
                    # hw: only one PSUM input per op, so copy then square
                    t16 = work.tile([128, 1024], f16, tag="t16", bufs=1,
                                    name=f"t16_{qc}_{kt}")
                    nc.vector.tensor_copy(t16[:], s_re[:])
                    nc.gpsimd.tensor_tensor(sqre[:], t16[:], t16[:],
                                            ALU.mult)
                sqim = work.tile([128, 1024], f16, tag="sqim", bufs=2,
                                 name=f"sqim_{qc}_{kt}")
                ti16 = work.tile([128, 1024], f16, tag="ti16", bufs=1,
                                 name=f"ti16_{qc}_{kt}")
                nc.vector.tensor_copy(ti16[:], s_im[:])
                if qc == 0:
                    # Pool mul keeps the prologue Act queue short
                    nc.gpsimd.tensor_tensor(sqim[:], ti16[:], ti16[:],
                                            ALU.mult)
                else:
                    nc.vector.tensor_mul(sqim[:], ti16[:], ti16[:])
                nc.gpsimd.tensor_tensor(bt[:, kb * 1024:(kb + 1) * 1024],
                                        sqre[:], sqim[:], ALU.add)

            def new_batch_tile(qc):
                return work.tile([128, 16 * 1024], f16, tag="batch", bufs=2,
                                 name=f"bt_{qc}")

            def new_half_tile(qc, half):
                return work.tile([128, 8 * 1024], f16, tag="batch3", bufs=2,
                                 name=f"bt_{qc}_{half}")

            def emit_sqrt(bt):
                nc.scalar.activation(bt[:], bt[:], AF.Sqrt,
                                     scale=1.0 / 64.0)

            def emit_av(bt, o_ps, qc, kts=range(KT), blk0=0):
                for kt in kts:
                    k8 = kt - blk0
                    for h in (0, 1):
                        col = slice(k8 * 1024 + h * 512,
                                    k8 * 1024 + h * 512 + 512)
                        vblk = v16_h[h][:, kt * 128:(kt + 1) * 128]
                        nc.tensor.matmul(o_ps[h][:, :], vblk, bt[:, col],
                                         start=(kt == 0),
                                         stop=(kt == KT - 1))

            def start_av(qc):
                return [psA.tile([128, QCW], f32, tag="oAV", bufs=2,
                                 name=f"o{h}_{qc}") for h in (0, 1)]

            def emit_norm(o_ps, paccs, qc):
                qs = slice(qc * QCW, (qc + 1) * QCW)
                rss = []
                for h in (0, 1):
                    rs = psA.tile([1, 512], f32, tag="s_im", bufs=1,
                                  name=f"rs_{qc}_{h}")
                    for pi, pc in enumerate(paccs):
                        nc.tensor.matmul(rs[:], ones16[:],
                                         pc[:, h * 512:h * 512 + 512],
                                         start=(pi == 0),
                                         stop=(pi == len(paccs) - 1))
                    rss.append(rs)
                bc_sb = work.tile([128, 1024], f32r, tag="bc_sb", bufs=1,
                                  name=f"bc_sb_{qc}")
                for h in (0, 1):
                    recip = work.tile([1, QCW], f32r, tag=f"recip{h}",
                                      bufs=1, name=f"recip{h}_{qc}")
                    nc.vector.reciprocal(recip[:], rss[h][:])
                    bc = psA.tile([128, 512], f32, tag="s_im", bufs=1,
                                  name=f"bc_{qc}_{h}")
                    nc.tensor.matmul(bc[:], ones_row[:], recip[:],
                                     start=True, stop=True)
                    nc.vector.tensor_copy(
                        bc_sb[:, h * 512:h * 512 + 512], bc[:])
                for h in (0, 1):
                    o_sb = work.tile([128, QCW], f16, tag="o_sb", bufs=1,
                                     name=f"o_sb_{qc}_{h}")
                    nc.scalar.copy(o_sb[:], o_ps[h][:])
                    for ri, dest in ((0, oT_re), (1, oT_im)):
                        rows = slice(64 * ri, 64 * ri + 64)
                        nc.vector.scalar_tensor_tensor(
                            dest[64 * h:64 * h + 64, qs],
                            o_sb[rows, :], 1.0,
                            bc_sb[rows, h * 512:h * 512 + 512],
                            ALU.mult, ALU.mult)

            def emit_wo(qc, rc_lo=0, rc_hi=4):
                qs = slice(qc * QCW, (qc + 1) * QCW)
                for Rc in range(rc_lo, rc_hi):
                    wo_re = psA.tile([128, QCW], f32, tag="oAV", bufs=2,
                                     name=f"wore_{Rc}_{qc}")
                    wo_im = psA.tile([128, QCW], f32, tag="oAV", bufs=2,
                                     name=f"woim_{Rc}_{qc}")

                    def wob(w):
                        return wop[:, w * 512 + Rc * 128:
                                   w * 512 + Rc * 128 + 128]

                    nc.tensor.matmul(wo_re[:], wob(0), oT_re[:, qs],
                                     start=True, stop=False)
                    nc.tensor.matmul(wo_re[:], wob(2), oT_im[:, qs],
                                     start=False, stop=True)
                    nc.tensor.matmul(wo_im[:], wob(1), oT_re[:, qs],
                                     start=True, stop=False)
                    nc.tensor.matmul(wo_im[:], wob(0), oT_im[:, qs],
                                     start=False, stop=True)
                    st_re = work.tile([128, QCW], f16, tag="st_re", bufs=1,
                                      name=f"st_re_{Rc}_{qc}")
                    nc.scalar.copy(st_re[:], wo_re[:])
                    nc.sync.dma_start(
                        ore_e[Rc * 128:(Rc + 1) * 128, qs], st_re[:])
                    st_im = work.tile([128, QCW], f16, tag="st_im", bufs=1,
                                      name=f"st_im_{Rc}_{qc}")
                    nc.scalar.copy(st_im[:], wo_im[:])
                    nc.sync.dma_start(
                        oim_e[Rc * 128:(Rc + 1) * 128, qs], st_im[:])

            def emit_exp(bt):
                nc.scalar.activation(bt[:], bt[:], AF.Exp, bias=eb_exp[:])

            def emit_pacc_half(qc, bt, a, blk0, eng=None, nblk=8):
                eng = eng or nc.vector
                pacc = work.tile([128, 1024], f16, tag=f"pacc{a}",
                                 bufs=1, name=f"pacc{a}_{qc}")
                eng.tensor_tensor(
                    pacc[:], bt[:, blk0 * 1024:(blk0 + 1) * 1024],
                    bt[:, (blk0 + 1) * 1024:(blk0 + 2) * 1024], ALU.add)
                for k8 in range(blk0 + 2, blk0 + nblk):
                    eng.tensor_tensor(
                        pacc[:], pacc[:],
                        bt[:, k8 * 1024:(k8 + 1) * 1024], ALU.add)
                return pacc

            def emit_pacc(qc, bt):
                # two independent accumulators so the rowsum matmul can
                # start after the first half
                return [emit_pacc_half(qc, bt, 0, 0, eng=nc.gpsimd),
                        emit_pacc_half(qc, bt, 1, 8, eng=nc.gpsimd)]

            # ---- emission script ----
            # prologue: A/C projections interleaved with chunk-0 scores and
            # the v projections, so every engine spins up early and the
            # steady-state iterations stay slim.
            dma_q(0)
            dma_q(1)
            bt0 = new_batch_tile(0)
            emit_proj_AC(0)
            emit_proj_AC(1)
            emit_proj_q(0)
            emit_proj_q(1)
            for kt in range(4):
                emit_scores_kt(0, kt, bt0)
            for j in range(2, NCC):
                emit_proj_AC(j)
                emit_scores_kt(0, 2 * j, bt0)
                emit_scores_kt(0, 2 * j + 1, bt0)
                if j >= 2:
                    emit_proj_v(j - 2)
                if j == 2:
                    dma_q(2)
                    dma_q(3)
            emit_sqrt(bt0)
            emit_exp(bt0)
            emit_proj_q(2)
            emit_proj_q(3)
            dma_q(4)
            dma_q(5)

            # steady pipeline: iter qc emits [WO(qc-2) fillers]
            # scores/sqrt/exp(qc), q-proj filler, then AV/pacc/norm(qc-1)
            # at the end (exp(qc-1) is finished well before).
            av_pend = (bt0, 0)     # exp'd, needs AV+pacc+norm
            wo_pend = None         # normalized, needs W_O
            for qc in range(1, QC):
                last = qc == QC - 1
                if last:
                    bth = [new_half_tile(qc, 0), new_half_tile(qc, 1)]
                else:
                    bt = new_batch_tile(qc)

                fillers = []
                if wo_pend is not None:
                    for rc in range(4):
                        fillers.append(
                            lambda rc=rc: emit_wo(wo_pend, rc, rc + 1))
                fi = 0
                for kt in range(KT):
                    emit_scores_kt(qc, kt,
                                   bth[kt // 8] if last else bt,
                                   (kt // 8) * 8 if last else 0)
                    if last and kt == 7:
                        emit_sqrt(bth[0])
                    if kt % 2 == 1 and fi < len(fillers):
                        fillers[fi]()
                        fi += 1
                while fi < len(fillers):
                    fillers[fi]()
                    fi += 1
                if last:
                    emit_sqrt(bth[1])
                else:
                    emit_sqrt(bt)
                    emit_exp(bt)
                if qc == 1:
                    emit_proj_v(6)
                    emit_proj_v(7)
                    emit_proj_q(4)
                    emit_proj_q(5)
                    dma_q(6)
                    dma_q(7)
                elif qc == 2:
                    emit_proj_q(6)
                    emit_proj_q(7)
                pbt, pqc2 = av_pend
                o_ps_av = start_av(pqc2)
                emit_av(pbt, o_ps_av, pqc2)
                paccs_new = emit_pacc(pqc2, pbt)
                emit_norm(o_ps_av, paccs_new, pqc2)
                wo_pend = pqc2
                if not last:
                    av_pend = (bt, qc)

            # drain chunk 3: exp/AV/pacc per half, then norm/WO
            emit_wo(wo_pend, 0, 4)
            o_ps3 = start_av(3)
            emit_exp(bth[0])
            emit_av(bth[0], o_ps3, 3, range(0, 8), 0)
            pacc3a = emit_pacc_half(3, bth[0], 0, 0)
            emit_exp(bth[1])
            emit_av(bth[1], o_ps3, 3, range(8, 16), 8)
            pacc3b = emit_pacc_half(3, bth[1], 1, 0, eng=nc.gpsimd, nblk=4)
            pacc3c = emit_pacc_half(3, bth[1], 2, 4, eng=nc.vector, nblk=4)
            emit_norm(o_ps3, [pacc3a, pacc3b, pacc3c], 3)
            emit_wo(3, 0, 4)

    nc.finalize()
    return nc



def _get_nc():
    if "nc" not in _CACHE:
        _CACHE["nc"] = _build_nc()
    return _CACHE["nc"]


def _core_inputs(c, inputs):
    b = c // 4
    h0 = 2 * (c % 4)
    hs = slice(h0 * 64, h0 * 64 + 128)

    def xpack_of(names):
        out = np.empty((NCC, 4 * len(names), 128, NCW), np.float16)
        for t, name in enumerate(names):
            xT = np.ascontiguousarray(inputs[name][b].T)      # (512, 2048)
            out[:, t * 4:(t + 1) * 4] = (
                xT.reshape(4, 128, NCC, NCW).transpose(2, 0, 1, 3))
        return out

    kx = xpack_of(("K_real", "K_imag"))
    qx = xpack_of(("Q_real", "Q_imag"))
    vx = xpack_of(("V_real", "V_imag"))

    wlist = []
    for kind in ("q", "A", "C"):
        base_r = inputs[{"q": "wq_r", "A": "wk_r", "C": "wk_r"}[kind]]
        base_i = inputs[{"q": "wq_i", "A": "wk_i", "C": "wk_i"}[kind]]
        for hh in (0, 1):
            rows = slice((h0 + hh) * 64, (h0 + hh) * 64 + 64)
            wr, wi_ = base_r[rows], base_i[rows]
            if kind == "C":
                w1 = np.vstack([-wi_, wr])       # x_re weights
                w2 = np.vstack([-wr, -wi_])      # x_im weights
            else:
                w1 = np.vstack([wr, wi_])
                w2 = np.vstack([-wi_, wr])
            wlist += [w1, w2]
    # v (transposed layout): moving weights [R, 128] = [wv_r.T | wv_i.T]
    # for the x_re passes and [-wv_i.T | wv_r.T] for the x_im passes.
    for hh in (0, 1):
        rows = slice((h0 + hh) * 64, (h0 + hh) * 64 + 64)
        wvr = inputs["wv_r"][rows]               # (64, 512)
        wvi = inputs["wv_i"][rows]
        W1 = np.concatenate([wvr.T, wvi.T], axis=1)      # (512, 128)
        W2 = np.concatenate([-wvi.T, wvr.T], axis=1)
        wlist += [W1.T, W2.T]                    # stored as (out, in)

    arr = np.empty((64, 128, 128), np.float16)
    for wi, mat in enumerate(wlist):
        arr[wi * 4:(wi + 1) * 4] = np.ascontiguousarray(mat.T).reshape(
            4, 128, 128)
    wpack = np.ascontiguousarray(arr.transpose(1, 0, 2)).reshape(
        128, 64 * 128)

    wo_r_T = np.ascontiguousarray(inputs["wo_r"][:, hs].T)    # (128, 512)
    wo_i_T = np.ascontiguousarray(inputs["wo_i"][:, hs].T)
    wopack = np.concatenate([wo_r_T, wo_i_T, -wo_i_T], axis=1)
    wopack = np.ascontiguousarray(wopack).astype(np.float16)

    return {
        "kx": kx,
        "qx": qx,
        "vx": vx,
        "wpack": wpack,
        "wopack": wopack,
        "onesr": np.ones((1, 128), np.float32),
    }


def kernel(**inputs):
    from concourse.bass_utils import run_bass_kernel_spmd

    nc = _get_nc()
    in_maps = [_core_inputs(c, inputs) for c in range(NCORES)]
    res = run_bass_kernel_spmd(nc, in_maps, list(range(NCORES)))
    out = np.empty((B, NQ, R, 2), np.float32)
    for b in range(B):
        re = np.zeros((512, NQ), np.float64)
        im = np.zeros((512, NQ), np.float64)
        for c in range(b * 4, b * 4 + 4):
            re += res.results[c]["out_re"].astype(np.float64)
            im += res.results[c]["out_im"].astype(np.float64)
        out[b, :, :, 0] = re.T
        out[b, :, :, 1] = im.T
    return out


# revision 51
# speedup vs baseline: 1.0373x; 1.0061x over previous
"""Complex-valued multi-head attention on 8 Trainium2 NeuronCores.

Sharding: batch(2) x head-pairs(4) -> 8 cores; each core runs one batch
element and 2 heads end-to-end (QKV proj -> complex scores -> |s| softmax
-> AV -> partial W_O), host sums the W_O partials over the 4 cores of each
batch element (tensor-parallel reduce) and transposes to the output layout.

All matmuls fp16 (fp32 PSUM).  K projections (A=[kr;ki], C=[-ki;kr]) run
first over all column blocks so chunk-0 scores unlock ~30us in; q/V
projections are emitted later as PE filler.  V is projected directly in
transposed [k,dv] layout by swapping matmul operands (x block stationary),
removing the PE transposes and their evacuation copies.  Per score tile
[128k x 1024]: Pool squares s_re (and some s_im) straight out of PSUM into
f16 SBUF, DVE squares the rest and does the f16 adds plus a running f16
accumulation of the probabilities - the rowsum is one ones^T @ pacc matmul
per chunk instead of 32 ones-column matmuls.  Act runs only the batched
sqrt and exp passes (one 16k-column batch per chunk; chunk 0's sqrt in two
halves for an earlier start), so table sets swap just twice per chunk.
AV/normalize/W_O for chunk i are emitted during chunk i+1; W_O partials
are stored and DMA'd in fp16 and summed on the host in float64.
"""
import sys

sys.path.insert(0, "/opt/trn_rl_repo")

import numpy as np

B, NQ, NK, R = 2, 2048, 2048, 512
H, DK, DV = 8, 64, 64
NCORES = 8
NCC = 8          # n-chunks for projection streaming (2048/256)
NCW = 256        # projection n-chunk width
QC = 4           # q-chunks in attention (2048/512)
QCW = 512
KT = 16          # k-tiles (2048/128)

_CACHE = {}


def _build_nc():
    import concourse.bass as bass
    import concourse.tile as tile
    from concourse import bacc, mybir

    f32 = mybir.dt.float32
    f32r = mybir.dt.float32r
    f16 = mybir.dt.float16
    ALU = mybir.AluOpType
    AF = mybir.ActivationFunctionType

    nc = bacc.Bacc("TRN2", target_bir_lowering=False, debug=False,
                   num_devices=NCORES)

    kx_e = nc.dram_tensor("kx", [NCC, 8, 128, NCW], f16,
                          kind="ExternalInput")
    qx_e = nc.dram_tensor("qx", [NCC, 8, 128, NCW], f16,
                          kind="ExternalInput")
    vx_e = nc.dram_tensor("vx", [NCC, 8, 128, NCW], f16,
                          kind="ExternalInput")
    wpack_e = nc.dram_tensor("wpack", [128, 64 * 128], f16,
                             kind="ExternalInput")
    wopack_e = nc.dram_tensor("wopack", [128, 3 * 512], f16,
                              kind="ExternalInput")
    onesr_e = nc.dram_tensor("onesr", [1, 128], f32r, kind="ExternalInput")
    ore_e = nc.dram_tensor("out_re", [512, NQ], f16, kind="ExternalOutput")
    oim_e = nc.dram_tensor("out_im", [512, NQ], f16, kind="ExternalOutput")

    with tile.TileContext(nc) as tc:
      with nc.allow_low_precision(reason="fp16 softmax path"):
        with tc.tile_pool(name="pers", bufs=1) as pers, \
             tc.tile_pool(name="work", bufs=2) as work, \
             tc.tile_pool(name="psA", bufs=1, space="PSUM") as psA:

            # ---- constants (A/C weight slice first so PE starts early) ----
            wp = pers.tile([128, 64 * 128], f16, tag="wp")
            AC_LO, AC_HI = 4 * 4 * 128, 12 * 4 * 128
            nc.sync.dma_start(wp[:, AC_LO:AC_HI], wpack_e[:, AC_LO:AC_HI])
            ones_row = pers.tile([1, 128], f32r, tag="ones_row")
            nc.sync.dma_start(ones_row[:], onesr_e[:])
            ones16 = pers.tile([128, 1], f16, tag="ones16")
            nc.vector.memset(ones16[:], 1.0)
            eb_exp = pers.tile([128, 1], f32, tag="eb_exp")
            nc.vector.memset(eb_exp[:], -1.5)          # exp(mag - 1.5)

            qxts = {}
            vxts = {}

            def dma_q(j):
                qxt = work.tile([128, 8 * NCW], f16, tag="qxt", bufs=1,
                                name=f"qxt_{j}")
                nc.sync.dma_start(
                    qxt[:].rearrange("p (b f) -> p b f", f=NCW),
                    qx_e[j].rearrange("b p f -> p b f"))
                qxts[j] = qxt

            def dma_v(j):
                vxt = work.tile([128, 8 * NCW], f16, tag="vxt", bufs=2,
                                name=f"vxt_{j}")
                nc.sync.dma_start(
                    vxt[:].rearrange("p (b f) -> p b f", f=NCW),
                    vx_e[j].rearrange("b p f -> p b f"))
                vxts[j] = vxt

            # k-block input DMAs up front
            kxts = []
            for j in range(NCC):
                if j == 1:
                    nc.sync.dma_start(wp[:, 0:AC_LO], wpack_e[:, 0:AC_LO])
                kxt = work.tile([128, 8 * NCW], f16, tag="kxt", bufs=2,
                                name=f"kxt_{j}")
                nc.sync.dma_start(
                    kxt[:].rearrange("p (b f) -> p b f", f=NCW),
                    kx_e[j].rearrange("b p f -> p b f"))
                kxts.append(kxt)
                if j == 2:
                    # v weights ride between the k blocks
                    nc.sync.dma_start(wp[:, AC_HI:], wpack_e[:, AC_HI:])
                if j >= 2:
                    dma_v(j - 2)
            dma_v(6)
            dma_v(7)
            wop = pers.tile([128, 3 * 512], f16, tag="wop")
            nc.sync.dma_start(wop[:], wopack_e[:])

            def wblk(w, rc):
                return wp[:, (w * 4 + rc) * 128:(w * 4 + rc + 1) * 128]

            # ---- projection destinations (fp16) ----
            q_sb = [pers.tile([128, NQ], f16, tag=f"q_sb{h}",
                              name=f"q_sb{h}") for h in (0, 1)]
            A_sb = [pers.tile([128, NK], f16, tag=f"A_sb{h}",
                              name=f"A_sb{h}") for h in (0, 1)]
            C_sb = [pers.tile([128, NK], f16, tag=f"C_sb{h}",
                              name=f"C_sb{h}") for h in (0, 1)]
            v16_h = [pers.tile([128, NK], f16, tag=f"v16_h{h}",
                               name=f"v16_h{h}") for h in (0, 1)]

            oT_re = pers.tile([128, NQ], f16, tag="oT_re")
            oT_im = pers.tile([128, NQ], f16, tag="oT_im")

            # round-robin engines for projection PSUM evacuation.
            # Act participates only while otherwise idle (the AC prologue);
            # later copies would jam the Act queue ahead of sqrt/exp.
            cp_engines = [[nc.scalar, nc.vector],
                          [nc.vector]]
            cp_state = [0, 0]

            def cp_copy(dst, src, phase=1):
                engs = cp_engines[phase]
                eng = engs[cp_state[phase] % len(engs)]
                cp_state[phase] += 1
                if eng is nc.scalar:
                    eng.copy(dst, src)
                else:
                    eng.tensor_copy(dst, src)

            def emit_proj_AC(j):
                kxt = kxts[j]
                cs = slice(j * NCW, (j + 1) * NCW)
                # dests A0,A1,C0,C1 <- weight blocks (4,5),(6,7),(8,9),(10,11)
                for di, dest in enumerate([A_sb[0], A_sb[1],
                                           C_sb[0], C_sb[1]]):
                    pj = psA.tile([128, NCW], f32, tag="s_re", bufs=2,
                                  name=f"pjk_{j}_{di}")
                    w0 = 4 + 2 * di
                    for rc in range(4):
                        nc.tensor.matmul(
                            pj[:], wblk(w0, rc),
                            kxt[:, rc * NCW:(rc + 1) * NCW],
                            start=(rc == 0), stop=False)
                    for rc in range(4):
                        nc.tensor.matmul(
                            pj[:], wblk(w0 + 1, rc),
                            kxt[:, (4 + rc) * NCW:(5 + rc) * NCW],
                            start=False, stop=(rc == 3))
                    cp_copy(dest[:, cs], pj[:], phase=0)

            def emit_proj_q(j):
                qxt = qxts.pop(j)
                cs = slice(j * NCW, (j + 1) * NCW)
                for hh, dest in enumerate(q_sb):
                    pj = psA.tile([128, NCW], f32, tag="s_re", bufs=2,
                                  name=f"pjq_{j}_{hh}")
                    for rc in range(4):
                        nc.tensor.matmul(
                            pj[:], wblk(2 * hh, rc),
                            qxt[:, rc * NCW:(rc + 1) * NCW],
                            start=(rc == 0), stop=False)
                    for rc in range(4):
                        nc.tensor.matmul(
                            pj[:], wblk(2 * hh + 1, rc),
                            qxt[:, (4 + rc) * NCW:(5 + rc) * NCW],
                            start=False, stop=(rc == 3))
                    cp_copy(dest[:, cs], pj[:])

            def emit_proj_v(j):
                vxt = vxts.pop(j)
                for h in (0, 1):
                    for nb in (0, 1):
                        vj = psA.tile([128, 128], f32, tag="oAV",
                                      bufs=2, name=f"vj_{j}_{h}_{nb}")
                        for rc in range(4):
                            c0 = rc * NCW + nb * 128
                            nc.tensor.matmul(vj[:], vxt[:, c0:c0 + 128],
                                             wblk(12 + 2 * h, rc),
                                             start=(rc == 0), stop=False)
                        for rc in range(4):
                            c0 = (4 + rc) * NCW + nb * 128
                            nc.tensor.matmul(vj[:], vxt[:, c0:c0 + 128],
                                             wblk(13 + 2 * h, rc),
                                             start=False, stop=(rc == 3))
                        kb = (2 * j + nb) * 128
                        cp_copy(v16_h[h][:, kb:kb + 128], vj[:])

            # ---- attention pieces ----
            def emit_scores_kt(qc, kt, bt, blk0=0):
                # GPSIMD cannot read PSUM: squares run on Act/DVE, the
                # f16 adds and pacc run on Pool (SBUF only).
                qs = slice(qc * QCW, (qc + 1) * QCW)
                ks = slice(kt * 128, (kt + 1) * 128)
                kb = kt - blk0
                s_re = psA.tile([128, 1024], f32, tag="s_re", bufs=2,
                                name=f"s_re_{qc}_{kt}")
                s_im = psA.tile([128, 1024], f32, tag="s_im", bufs=1,
                                name=f"s_im_{qc}_{kt}")
                for h in (0, 1):
                    col = slice(h * 512, h * 512 + 512)
                    nc.tensor.matmul(s_re[:, col], A_sb[h][:, ks],
                                     q_sb[h][:, qs], start=True, stop=True)
                    nc.tensor.matmul(s_im[:, col], C_sb[h][:, ks],
                                     q_sb[h][:, qs], start=True, stop=True)
                sqre = work.tile([128, 1024], f16, tag="sqre", bufs=2,
                                 name=f"sqre_{qc}_{kt}")
                if qc == 0:
                    # Act squares hide inside the projection prologue
                    nc.scalar.square(sqre[:], s_re[:])
                else:
                    # hw: only one PSUM input per op, so copy then square
                    t16 = work.tile([128, 1024], f16, tag="t16", bufs=1,
                                    name=f"t16_{qc}_{kt}")
                    nc.vector.tensor_copy(t16[:], s_re[:])
                    nc.gpsimd.tensor_tensor(sqre[:], t16[:], t16[:],
                                            ALU.mult)
                sqim = work.tile([128, 1024], f16, tag="sqim", bufs=2,
                                 name=f"sqim_{qc}_{kt}")
                ti16 = work.tile([128, 1024], f16, tag="ti16", bufs=1,
                                 name=f"ti16_{qc}_{kt}")
                nc.vector.tensor_copy(ti16[:], s_im[:])
                if qc == 0:
                    # Pool mul keeps the prologue Act queue short
                    nc.gpsimd.tensor_tensor(sqim[:], ti16[:], ti16[:],
                                            ALU.mult)
                else:
                    nc.vector.tensor_mul(sqim[:], ti16[:], ti16[:])
                nc.gpsimd.tensor_tensor(bt[:, kb * 1024:(kb + 1) * 1024],
                                        sqre[:], sqim[:], ALU.add)

            def new_batch_tile(qc):
                return work.tile([128, 16 * 1024], f16, tag="batch", bufs=2,
                                 name=f"bt_{qc}")

            def new_half_tile(qc, half):
                return work.tile([128, 8 * 1024], f16, tag="batch3", bufs=2,
                                 name=f"bt_{qc}_{half}")

            def emit_sqrt(bt):
                nc.scalar.activation(bt[:], bt[:], AF.Sqrt,
                                     scale=1.0 / 64.0)

            def emit_av(bt, o_ps, qc, kts=range(KT), blk0=0):
                for kt in kts:
                    k8 = kt - blk0
                    for h in (0, 1):
                        col = slice(k8 * 1024 + h * 512,
                                    k8 * 1024 + h * 512 + 512)
                        vblk = v16_h[h][:, kt * 128:(kt + 1) * 128]
                        nc.tensor.matmul(o_ps[h][:, :], vblk, bt[:, col],
                                         start=(kt == 0),
                                         stop=(kt == KT - 1))

            def start_av(qc):
                return [psA.tile([128, QCW], f32, tag="oAV", bufs=2,
                                 name=f"o{h}_{qc}") for h in (0, 1)]

            def emit_norm(o_ps, paccs, qc):
                qs = slice(qc * QCW, (qc + 1) * QCW)
                rss = []
                for h in (0, 1):
                    rs = psA.tile([1, 512], f32, tag="s_im", bufs=1,
                                  name=f"rs_{qc}_{h}")
                    for pi, pc in enumerate(paccs):
                        nc.tensor.matmul(rs[:], ones16[:],
                                         pc[:, h * 512:h * 512 + 512],
                                         start=(pi == 0),
                                         stop=(pi == len(paccs) - 1))
                    rss.append(rs)
                bc_sb = work.tile([128, 1024], f32r, tag="bc_sb", bufs=1,
                                  name=f"bc_sb_{qc}")
                for h in (0, 1):
                    recip = work.tile([1, QCW], f32r, tag=f"recip{h}",
                                      bufs=1, name=f"recip{h}_{qc}")
                    nc.vector.reciprocal(recip[:], rss[h][:])
                    bc = psA.tile([128, 512], f32, tag="s_im", bufs=1,
                                  name=f"bc_{qc}_{h}")
                    nc.tensor.matmul(bc[:], ones_row[:], recip[:],
                                     start=True, stop=True)
                    nc.vector.tensor_copy(
                        bc_sb[:, h * 512:h * 512 + 512], bc[:])
                for h in (0, 1):
                    o_sb = work.tile([128, QCW], f16, tag="o_sb", bufs=1,
                                     name=f"o_sb_{qc}_{h}")
                    nc.scalar.copy(o_sb[:], o_ps[h][:])
                    for ri, dest in ((0, oT_re), (1, oT_im)):
                        rows = slice(64 * ri, 64 * ri + 64)
                        nc.gpsimd.tensor_tensor(
                            dest[64 * h:64 * h + 64, qs],
                            o_sb[rows, :],
                            bc_sb[rows, h * 512:h * 512 + 512],
                            ALU.mult)

            def emit_wo(qc, rc_lo=0, rc_hi=4):
                qs = slice(qc * QCW, (qc + 1) * QCW)
                for Rc in range(rc_lo, rc_hi):
                    wo_re = psA.tile([128, QCW], f32, tag="oAV", bufs=2,
                                     name=f"wore_{Rc}_{qc}")
                    wo_im = psA.tile([128, QCW], f32, tag="oAV", bufs=2,
                                     name=f"woim_{Rc}_{qc}")

                    def wob(w):
                        return wop[:, w * 512 + Rc * 128:
                                   w * 512 + Rc * 128 + 128]

                    nc.tensor.matmul(wo_re[:], wob(0), oT_re[:, qs],
                                     start=True, stop=False)
                    nc.tensor.matmul(wo_re[:], wob(2), oT_im[:, qs],
                                     start=False, stop=True)
                    nc.tensor.matmul(wo_im[:], wob(1), oT_re[:, qs],
                                     start=True, stop=False)
                    nc.tensor.matmul(wo_im[:], wob(0), oT_im[:, qs],
                                     start=False, stop=True)
                    st_re = work.tile([128, QCW], f16, tag="st_re", bufs=1,
                                      name=f"st_re_{Rc}_{qc}")
                    nc.scalar.copy(st_re[:], wo_re[:])
                    nc.sync.dma_start(
                        ore_e[Rc * 128:(Rc + 1) * 128, qs], st_re[:])
                    st_im = work.tile([128, QCW], f16, tag="st_im", bufs=1,
                                      name=f"st_im_{Rc}_{qc}")
                    nc.scalar.copy(st_im[:], wo_im[:])
                    nc.sync.dma_start(
                        oim_e[Rc * 128:(Rc + 1) * 128, qs], st_im[:])

            def emit_exp(bt):
                nc.scalar.activation(bt[:], bt[:], AF.Exp, bias=eb_exp[:])

            def emit_pacc_half(qc, bt, a, blk0, eng=None, nblk=8):
                eng = eng or nc.vector
                pacc = work.tile([128, 1024], f16, tag=f"pacc{a}",
                                 bufs=1, name=f"pacc{a}_{qc}")
                eng.tensor_tensor(
                    pacc[:], bt[:, blk0 * 1024:(blk0 + 1) * 1024],
                    bt[:, (blk0 + 1) * 1024:(blk0 + 2) * 1024], ALU.add)
                for k8 in range(blk0 + 2, blk0 + nblk):
                    eng.tensor_tensor(
                        pacc[:], pacc[:],
                        bt[:, k8 * 1024:(k8 + 1) * 1024], ALU.add)
                return pacc

            def emit_pacc(qc, bt):
                # two independent accumulators so the rowsum matmul can
                # start after the first half
                return [emit_pacc_half(qc, bt, 0, 0, eng=nc.gpsimd),
                        emit_pacc_half(qc, bt, 1, 8, eng=nc.gpsimd)]

            # ---- emission script ----
            # prologue: A/C projections interleaved with chunk-0 scores and
            # the v projections, so every engine spins up early and the
            # steady-state iterations stay slim.
            dma_q(0)
            dma_q(1)
            bt0 = new_batch_tile(0)
            emit_proj_AC(0)
            emit_proj_AC(1)
            emit_proj_q(0)
            emit_proj_q(1)
            for kt in range(4):
                emit_scores_kt(0, kt, bt0)
            for j in range(2, NCC):
                emit_proj_AC(j)
                emit_scores_kt(0, 2 * j, bt0)
                emit_scores_kt(0, 2 * j + 1, bt0)
                if j >= 2:
                    emit_proj_v(j - 2)
                if j == 2:
                    dma_q(2)
                    dma_q(3)
            emit_sqrt(bt0)
            emit_exp(bt0)
            emit_proj_q(2)
            emit_proj_q(3)
            dma_q(4)
            dma_q(5)

            # steady pipeline: iter qc emits [WO(qc-2) fillers]
            # scores/sqrt/exp(qc), q-proj filler, then AV/pacc/norm(qc-1)
            # at the end (exp(qc-1) is finished well before).
            av_pend = (bt0, 0)     # exp'd, needs AV+pacc+norm
            wo_pend = None         # normalized, needs W_O
            for qc in range(1, QC):
                last = qc == QC - 1
                if last:
                    bth = [new_half_tile(qc, 0), new_half_tile(qc, 1)]
                else:
                    bt = new_batch_tile(qc)

                fillers = []
                if wo_pend is not None:
                    for rc in range(4):
                        fillers.append(
                            lambda rc=rc: emit_wo(wo_pend, rc, rc + 1))
                fi = 0
                for kt in range(KT):
                    emit_scores_kt(qc, kt,
                                   bth[kt // 8] if last else bt,
                                   (kt // 8) * 8 if last else 0)
                    if last and kt == 7:
                        emit_sqrt(bth[0])
                    if kt % 2 == 1 and fi < len(fillers):
                        fillers[fi]()
                        fi += 1
                while fi < len(fillers):
                    fillers[fi]()
                    fi += 1
                if last:
                    emit_sqrt(bth[1])
                else:
                    emit_sqrt(bt)
                    emit_exp(bt)
                if qc == 1:
                    emit_proj_v(6)
                    emit_proj_v(7)
                    emit_proj_q(4)
                    emit_proj_q(5)
                    dma_q(6)
                    dma_q(7)
                elif qc == 2:
                    emit_proj_q(6)
                    emit_proj_q(7)
                pbt, pqc2 = av_pend
                o_ps_av = start_av(pqc2)
                emit_av(pbt, o_ps_av, pqc2)
                paccs_new = emit_pacc(pqc2, pbt)
                emit_norm(o_ps_av, paccs_new, pqc2)
                wo_pend = pqc2
                if not last:
                    av_pend = (bt, qc)

            # drain chunk 3: exp/AV/pacc per half, then norm/WO
            emit_wo(wo_pend, 0, 4)
            o_ps3 = start_av(3)
            emit_exp(bth[0])
            emit_av(bth[0], o_ps3, 3, range(0, 8), 0)
            pacc3a = emit_pacc_half(3, bth[0], 0, 0)
            emit_exp(bth[1])
            emit_av(bth[1], o_ps3, 3, range(8, 16), 8)
            pacc3b = emit_pacc_half(3, bth[1], 1, 0, eng=nc.gpsimd, nblk=4)
            pacc3c = emit_pacc_half(3, bth[1], 2, 4, eng=nc.vector, nblk=4)
            emit_norm(o_ps3, [pacc3a, pacc3b, pacc3c], 3)
            emit_wo(3, 0, 4)

    nc.finalize()
    return nc



def _get_nc():
    if "nc" not in _CACHE:
        _CACHE["nc"] = _build_nc()
    return _CACHE["nc"]


def _core_inputs(c, inputs):
    b = c // 4
    h0 = 2 * (c % 4)
    hs = slice(h0 * 64, h0 * 64 + 128)

    def xpack_of(names):
        out = np.empty((NCC, 4 * len(names), 128, NCW), np.float16)
        for t, name in enumerate(names):
            xT = np.ascontiguousarray(inputs[name][b].T)      # (512, 2048)
            out[:, t * 4:(t + 1) * 4] = (
                xT.reshape(4, 128, NCC, NCW).transpose(2, 0, 1, 3))
        return out

    kx = xpack_of(("K_real", "K_imag"))
    qx = xpack_of(("Q_real", "Q_imag"))
    vx = xpack_of(("V_real", "V_imag"))

    wlist = []
    for kind in ("q", "A", "C"):
        base_r = inputs[{"q": "wq_r", "A": "wk_r", "C": "wk_r"}[kind]]
        base_i = inputs[{"q": "wq_i", "A": "wk_i", "C": "wk_i"}[kind]]
        for hh in (0, 1):
            rows = slice((h0 + hh) * 64, (h0 + hh) * 64 + 64)
            wr, wi_ = base_r[rows], base_i[rows]
            if kind == "C":
                w1 = np.vstack([-wi_, wr])       # x_re weights
                w2 = np.vstack([-wr, -wi_])      # x_im weights
            else:
                w1 = np.vstack([wr, wi_])
                w2 = np.vstack([-wi_, wr])
            wlist += [w1, w2]
    # v (transposed layout): moving weights [R, 128] = [wv_r.T | wv_i.T]
    # for the x_re passes and [-wv_i.T | wv_r.T] for the x_im passes.
    for hh in (0, 1):
        rows = slice((h0 + hh) * 64, (h0 + hh) * 64 + 64)
        wvr = inputs["wv_r"][rows]               # (64, 512)
        wvi = inputs["wv_i"][rows]
        W1 = np.concatenate([wvr.T, wvi.T], axis=1)      # (512, 128)
        W2 = np.concatenate([-wvi.T, wvr.T], axis=1)
        wlist += [W1.T, W2.T]                    # stored as (out, in)

    arr = np.empty((64, 128, 128), np.float16)
    for wi, mat in enumerate(wlist):
        arr[wi * 4:(wi + 1) * 4] = np.ascontiguousarray(mat.T).reshape(
            4, 128, 128)
    wpack = np.ascontiguousarray(arr.transpose(1, 0, 2)).reshape(
        128, 64 * 128)

    wo_r_T = np.ascontiguousarray(inputs["wo_r"][:, hs].T)    # (128, 512)
    wo_i_T = np.ascontiguousarray(inputs["wo_i"][:, hs].T)
    wopack = np.concatenate([wo_r_T, wo_i_T, -wo_i_T], axis=1)
    wopack = np.ascontiguousarray(wopack).astype(np.float16)

    return {
        "kx": kx,
        "qx": qx,
        "vx": vx,
        "wpack": wpack,
        "wopack": wopack,
        "onesr": np.ones((1, 128), np.float32),
    }


def kernel(**inputs):
    from concourse.bass_utils import run_bass_kernel_spmd

    nc = _get_nc()
    in_maps = [_core_inputs(c, inputs) for c in range(NCORES)]
    res = run_bass_kernel_spmd(nc, in_maps, list(range(NCORES)))
    out = np.empty((B, NQ, R, 2), np.float32)
    for b in range(B):
        re = np.zeros((512, NQ), np.float64)
        im = np.zeros((512, NQ), np.float64)
        for c in range(b * 4, b * 4 + 4):
            re += res.results[c]["out_re"].astype(np.float64)
            im += res.results[c]["out_im"].astype(np.float64)
        out[b, :, :, 0] = re.T
        out[b, :, :, 1] = im.T
    return out
